# revision 5
# baseline (speedup 1.0000x reference)
"""ConvAttention Trainium2 kernel.

Full-input contract: kernel(**inputs) takes the complete unsharded inputs
(x: (8, 512, 32, 32), gamma: (1, 512, 1, 1), w_qkv: (1536, 512),
w_out: (512, 512)) and returns the full (8, 512, 32, 32) output.

Sharding: data-parallel over batch — core b computes batch element b
entirely on-chip. No collectives.

Per-core math (b fixed), [channel(part), spatial(free)] layout:
  xn = LayerNorm_c(x) * gamma          (stats via ones-matmul rows on PE)
  q,k = W_qk @ xn ; vT = xn^T W_v^T    (vT quantized to fp8e4, DoubleRow
                                        pair layout [j,pair,h,sub,80])
  per head: simT = k^T q (bf16); expT = fp8e4(exp(simT*s - 4ln2))
            (bias cancels in normalization; max exp ~25 << 240)
  out_aug = DoubleRow fp8 matmul [v;1] @ exp -> numerator + denominator
  att = numerator * broadcast(1/denominator)
  y = w_out @ att + x

Structure tuned against the HAM clock gate: exp on ACT (1.2 GHz fixed) is
the pacing engine, so ACT runs exp ONLY (evictions on DVE), the qk
projection is folded into the per-head pipeline to keep PE dense, and the
attn@v matmul runs fp8 DoubleRow (half the PE stream cycles of bf16).
"""

import numpy as np

C = 512
N = 1024
O3 = 1536
H = 8
DH = 64
EPS = 1e-5
SCALE = 64.0 ** -0.5
EXP_BIAS = -2.7725887222397811  # -4*ln2, cancels in softmax normalization
NCORES = 8

_CACHE = {}


def _build():
    import concourse.bacc as bacc
    import concourse.tile as tile
    from concourse import mybir
    from concourse.masks import make_identity

    f32 = mybir.dt.float32
    f32r = mybir.dt.float32r
    bf16 = mybir.dt.bfloat16
    fp8 = mybir.dt.float8e4
    AF = mybir.ActivationFunctionType
    OP = mybir.AluOpType
    DR = mybir.MatmulPerfMode.DoubleRow

    nc = bacc.Bacc("TRN2", target_bir_lowering=False, debug=False, num_devices=1)
    x_ap = nc.dram_tensor("x", [C, N], f32, kind="ExternalInput").ap()
    g_ap = nc.dram_tensor("gamma", [C], f32, kind="ExternalInput").ap()
    wqkv_ap = nc.dram_tensor("w_qkv", [O3, C], f32, kind="ExternalInput").ap()
    wout_ap = nc.dram_tensor("w_out", [C, C], f32, kind="ExternalInput").ap()
    y_ap = nc.dram_tensor("y", [C, N], f32, kind="ExternalOutput").ap()

    mm = nc.tensor.matmul

    with tile.TileContext(nc) as tc:
        with (
            tc.tile_pool(name="const", bufs=1) as const,
            tc.tile_pool(name="xin", bufs=1) as xin,
            tc.tile_pool(name="acts", bufs=1) as acts,
            tc.tile_pool(name="wTp", bufs=1) as wTp,
            tc.tile_pool(name="rows", bufs=1) as rows,
        ):
            ident = const.tile([128, 128], f32)
            make_identity(nc, ident)
            ones_f = const.tile([128, 1], f32)
            nc.vector.memset(ones_f, 1.0)
            ones_col = const.tile([128, 1], f32r)
            nc.scalar.copy(out=ones_col, in_=ones_f)
            # row operand for K=1 broadcast matmuls (base partition 0 only)
            onesr_f = const.tile([1, DH], f32)
            nc.vector.memset(onesr_f, 1.0)
            ones_row = const.tile([1, DH], f32r)
            nc.scalar.copy(out=ones_row, in_=onesr_f)
            gamma_f = const.tile([1, C], f32)
            nc.sync.dma_start(out=gamma_f, in_=g_ap[None, :])
            gamma_row = const.tile([1, C], f32r)
            nc.scalar.copy(out=gamma_row, in_=gamma_f)
            eps_col = const.tile([65, 1], f32)
            nc.vector.memset(eps_col, EPS)
            bias_col = const.tile([128, 1], f32)
            nc.vector.memset(bias_col, EXP_BIAS)

            # ---- load x ----
            x_sb = xin.tile([128, 4, N], f32)
            for t in range(4):
                nc.sync.dma_start(
                    out=x_sb[:, t, :], in_=x_ap[t * 128 : (t + 1) * 128, :]
                )

            # ---- persistent activation tiles ----
            qk_sb = acts.tile([128, 8, N], bf16)  # q: 0..3, k: 4..7
            # v^T in DoubleRow pair layout: [j, jt-pair, h, sub, c-aug(80)]
            # cols 0:64 = v, col 64 = 1 (denominator), 65:80 = 0 (pad)
            vT2 = acts.tile([128, 4, H, 2, 80], fp8)
            att_sb = acts.tile([128, 4, N], bf16)
            nc.gpsimd.memset(vT2, 0.0)
            nc.gpsimd.memset(vT2[:, :, :, :, DH : DH + 1], 1.0)

            # weights, transposed ([contraction-part, out-free])
            wqkT = wTp.tile([128, 4, 1024], bf16)
            wvT = wTp.tile([128, 4, C], bf16)
            woT = wTp.tile([128, 4, C], bf16)

            # stat rows, packed at 32-aligned partitions of shared tiles
            stA = rows.tile([97, N], f32)  # mean@0, msq@32, var@64, sd@96
            stB = rows.tile([97, N], f32)  # a@0, b@32
            st_ra = rows.tile([1, N], f32r)  # a (rounded), base 0
            st_rb = rows.tile([1, N], f32r)  # b (rounded), base 0
            mean_r, msq_r, var_r, sd_r = (
                stA[0:1, :],
                stA[32:33, :],
                stA[64:65, :],
                stA[96:97, :],
            )
            a_r, b_r = stB[0:1, :], stB[32:33, :]

            with tc.tile_pool(name="xnp", bufs=1) as xnp:
                xn_sb = xnp.tile([128, 4, N], bf16)

                # ============ phase A: W transpose + stats + xn ============
                with (
                    tc.tile_pool(name="wnat", bufs=3) as wnat,
                    tc.tile_pool(name="tmp", bufs=2) as tmp,
                    tc.tile_pool(name="st_ps", bufs=1, space="PSUM") as st_ps,
                ):
                    st_px = st_ps.tile([1, N], f32, tag="sx")
                    st_pq = st_ps.tile([1, N], f32, tag="sq")

                    # ---- stats rows: sum(x) fp32 matmul, sum(x^2) f32r ----
                    for t in range(4):
                        xsq = tmp.tile([128, N], f32r, tag="xsq")
                        nc.vector.tensor_mul(xsq, x_sb[:, t, :], x_sb[:, t, :])
                        for ch in range(2):
                            sl = slice(ch * 512, (ch + 1) * 512)
                            mm(
                                st_px[:, sl],
                                ones_f,
                                x_sb[:, t, sl],
                                start=(t == 0),
                                stop=(t == 3),
                            )
                            mm(
                                st_pq[:, sl],
                                ones_col,
                                xsq[:, sl],
                                start=(t == 0),
                                stop=(t == 3),
                            )

                    with tc.tile_pool(name="tp_ps", bufs=2, space="PSUM") as tp_ps:
                        # evictions split DVE/ACT to balance phase-A load
                        tcount = [0]

                        def transp(dst, src):
                            ps = tp_ps.tile([128, 128], f32)
                            nc.tensor.transpose(ps, src, ident)
                            if tcount[0] % 2 == 0:
                                nc.vector.tensor_copy(dst, ps)
                            else:
                                nc.scalar.copy(out=dst, in_=ps)
                            tcount[0] += 1

                        for ot in range(12):
                            wn = wnat.tile([128, C], f32, tag="wn")
                            nc.sync.dma_start(
                                out=wn, in_=wqkv_ap[ot * 128 : (ot + 1) * 128, :]
                            )
                            for kt in range(4):
                                src = wn[:, kt * 128 : (kt + 1) * 128]
                                if ot < 8:
                                    transp(
                                        wqkT[:, kt, ot * 128 : (ot + 1) * 128], src
                                    )
                                else:
                                    transp(
                                        wvT[:, kt, (ot - 8) * 128 : (ot - 7) * 128],
                                        src,
                                    )
                        for ot in range(4):
                            wn = wnat.tile([128, C], f32, tag="wn")
                            nc.sync.dma_start(
                                out=wn, in_=wout_ap[ot * 128 : (ot + 1) * 128, :]
                            )
                            for kt in range(4):
                                transp(
                                    woT[:, kt, ot * 128 : (ot + 1) * 128],
                                    wn[:, kt * 128 : (kt + 1) * 128],
                                )

                    nc.scalar.activation(mean_r, st_px, AF.Copy, scale=1.0 / C)
                    nc.vector.tensor_mul(msq_r, mean_r, mean_r)
                    nc.vector.scalar_tensor_tensor(
                        out=var_r,
                        in0=st_pq,
                        scalar=1.0 / C,
                        in1=msq_r,
                        op0=OP.mult,
                        op1=OP.subtract,
                    )
                    nc.scalar.activation(
                        sd_r, var_r, AF.Sqrt, bias=eps_col[64:65, :]
                    )
                    nc.vector.reciprocal(a_r, sd_r)
                    nc.vector.scalar_tensor_tensor(
                        out=b_r,
                        in0=mean_r,
                        scalar=-1.0,
                        in1=a_r,
                        op0=OP.mult,
                        op1=OP.mult,
                    )
                    nc.scalar.copy(out=st_ra, in_=a_r)
                    nc.scalar.copy(out=st_rb, in_=b_r)

                    # ---- xn = x * (gamma (x) a) + (gamma (x) b) ----
                    with tc.tile_pool(name="ab_ps", bufs=1, space="PSUM") as ab_ps:
                        for t in range(4):
                            ab = ab_ps.tile([128, 2, N], f32)
                            tsl = slice(t * 128, (t + 1) * 128)
                            for ch in range(2):
                                sl = slice(ch * 512, (ch + 1) * 512)
                                mm(ab[:, 0, sl], gamma_row[0:1, tsl], st_ra[:, sl])
                                mm(ab[:, 1, sl], gamma_row[0:1, tsl], st_rb[:, sl])
                            for ch in range(2):
                                sl = slice(ch * 512, (ch + 1) * 512)
                                t1 = tmp.tile([128, 512], f32, tag="xnt")
                                nc.vector.tensor_mul(
                                    t1, x_sb[:, t, sl], ab[:, 0, sl]
                                )
                                nc.vector.tensor_add(
                                    xn_sb[:, t, sl], t1, ab[:, 1, sl]
                                )

                # ======= phase BC: fused qkv projection + attention =======
                # Per-head pipeline: sim/exp(h) | attnv+normalize(h-1), with
                # qk projections and the v projection interleaved to keep
                # the PE stream dense (HAM stays un-throttled). ACT runs exp
                # only; DVE takes every PSUM eviction.
                with (
                    tc.tile_pool(name="expT", bufs=2) as expp,
                    tc.tile_pool(name="rrp", bufs=2) as rrp,
                    tc.tile_pool(name="oanp", bufs=2) as oanp,
                    tc.tile_pool(name="sim_ps", bufs=2, space="PSUM") as sim_ps,
                    tc.tile_pool(name="qk_ps", bufs=1, space="PSUM") as qk_psp,
                    tc.tile_pool(name="oa_ps", bufs=1, space="PSUM") as oa_psp,
                    tc.tile_pool(name="rb_ps", bufs=1, space="PSUM") as rb_psp,
                ):

                    def qkproj_group(ot, ch):
                        sl = slice(ch * 512, (ch + 1) * 512)
                        ps = qk_psp.tile([128, 512], f32, tag="qk")
                        for kt in range(4):
                            mm(
                                ps,
                                wqkT[:, kt, ot * 128 : (ot + 1) * 128],
                                xn_sb[:, kt, sl],
                                start=(kt == 0),
                                stop=(kt == 3),
                            )
                        nc.vector.tensor_copy(qk_sb[:, ot, sl], ps)

                    def qkproj(pair):  # q,k o-tiles for heads 2p, 2p+1
                        for ot in (pair, 4 + pair):
                            for ch in range(2):
                                qkproj_group(ot, ch)

                    def vproj(nt):
                        ps = qk_psp.tile([128, 512], f32, tag="qk")
                        for kt in range(4):
                            mm(
                                ps,
                                xn_sb[:, kt, nt * 128 : (nt + 1) * 128],
                                wvT[:, kt, :],
                                start=(kt == 0),
                                stop=(kt == 3),
                            )
                        nc.vector.tensor_copy(
                            vT2[:, nt // 2, :, nt % 2, 0:DH],
                            ps.rearrange("p (h d) -> p h d", h=H),
                        )

                    def sim_exp(h, e_t, jts):
                        hp = (h % 2) * DH
                        q_h = qk_sb[hp : hp + DH, h // 2, :]
                        k_h = qk_sb[hp : hp + DH, 4 + h // 2, :]
                        for jt in jts:
                            sim_t = sim_ps.tile([128, N], f32, tag="sim")
                            for ch in range(2):
                                sl = slice(ch * 512, (ch + 1) * 512)
                                mm(
                                    sim_t[:, sl],
                                    k_h[:, jt * 128 : (jt + 1) * 128],
                                    q_h[:, sl],
                                )
                            nc.scalar.activation(
                                e_t[:, jt // 2, jt % 2, :],
                                sim_t,
                                AF.Exp,
                                scale=SCALE,
                                bias=bias_col,
                            )

                    def attnv(hm, e_t):
                        oa = oa_psp.tile([128, N], f32, tag="oa")
                        for p in range(4):
                            for ch in range(2):
                                sl = slice(ch * 512, (ch + 1) * 512)
                                mm(
                                    oa[0:80, sl],
                                    vT2[:, p, hm, :, :],
                                    e_t[:, p, :, sl],
                                    start=(p == 0),
                                    stop=(p == 3),
                                    perf_mode=DR,
                                )
                        # reciprocal_approx_fast misreads PSUM operands:
                        # stage the denominator row through SBUF first.
                        rden_t = rrp.tile([1, N], f32, tag="rden")
                        nc.vector.tensor_copy(rden_t, oa[DH : DH + 1, :])
                        rrec_t = rrp.tile([1, N], f32, tag="rr")
                        nc.vector.reciprocal_approx_fast(out=rrec_t, in_=rden_t)
                        rd = rrp.tile([1, N], f32r, tag="rd")
                        nc.vector.tensor_copy(rd, rrec_t)
                        return oa, rd

                    def normalize(h2, oa, rd):
                        # DVE has a single PSUM read port: numerator must be
                        # in SBUF before the mult against the PSUM rb rows.
                        hp2 = (h2 % 2) * DH
                        oan = oanp.tile([DH, N], f32, tag="oan")
                        nc.vector.tensor_copy(oan, oa[0:DH, :])
                        for ch in range(2):
                            sl = slice(ch * 512, (ch + 1) * 512)
                            rb = rb_psp.tile([DH, 512], f32, tag="rb")
                            mm(rb, ones_row, rd[:, sl])
                            nc.vector.tensor_mul(
                                att_sb[hp2 : hp2 + DH, h2 // 2, sl],
                                oan[:, sl],
                                rb,
                            )

                    # ---- fill: head 0 sims interleaved with v projection ----
                    qkproj(0)
                    e_prev = expp.tile([128, 4, 2, N], fp8, tag="exp")
                    for jt in range(8):
                        sim_exp(0, e_prev, [jt])
                        vproj(jt)
                    qkproj(1)

                    # ---- steady state ----
                    for it in range(1, H + 1):
                        h, hm = it, it - 1
                        e_t = None
                        if h < H:
                            e_t = expp.tile([128, 4, 2, N], fp8, tag="exp")
                            sim_exp(h, e_t, [0, 1, 2, 3])
                        oa, rd = attnv(hm, e_prev)
                        if h < H:
                            sim_exp(h, e_t, [4, 5, 6, 7])
                            if h == 2:
                                qkproj(2)
                            elif h == 4:
                                qkproj(3)
                        normalize(hm, oa, rd)
                        e_prev = e_t

            # ============ phase D: out projection + residual ============
            with (
                tc.tile_pool(name="evict", bufs=3) as evict,
                tc.tile_pool(name="z_ps", bufs=3, space="PSUM") as z_ps,
            ):
                for ot in range(4):
                    for ch in range(2):
                        sl = slice(ch * 512, (ch + 1) * 512)
                        ps = z_ps.tile([128, 512], f32)
                        for kt in range(4):
                            mm(
                                ps,
                                woT[:, kt, ot * 128 : (ot + 1) * 128],
                                att_sb[:, kt, sl],
                                start=(kt == 0),
                                stop=(kt == 3),
                            )
                        yt = evict.tile([128, 512], f32)
                        nc.vector.tensor_add(yt, ps, x_sb[:, ot, sl])
                        nc.sync.dma_start(
                            out=y_ap[ot * 128 : (ot + 1) * 128, sl], in_=yt
                        )

    nc.compile()
    return nc


def _get_nc():
    if "nc" not in _CACHE:
        _CACHE["nc"] = _build()
    return _CACHE["nc"]


def kernel(x, gamma, w_qkv, w_out):
    from concourse.bass_utils import run_bass_kernel_spmd

    x = np.ascontiguousarray(x, dtype=np.float32)
    gamma_f = np.ascontiguousarray(gamma, dtype=np.float32).reshape(C)
    wq = np.ascontiguousarray(w_qkv, dtype=np.float32)
    wo = np.ascontiguousarray(w_out, dtype=np.float32)

    nc = _get_nc()
    in_maps = [
        {
            "x": x[b].reshape(C, N).copy(),
            "gamma": gamma_f,
            "w_qkv": wq,
            "w_out": wo,
        }
        for b in range(NCORES)
    ]
    res = run_bass_kernel_spmd(nc, in_maps, core_ids=list(range(NCORES)))
    out = np.stack(
        [res.results[b]["y"].reshape(C, 32, 32) for b in range(NCORES)], axis=0
    )
    return out.astype(np.float32)


# revision 13
# speedup vs baseline: 1.0305x; 1.0305x over previous
"""ConvAttention Trainium2 kernel.

Full-input contract: kernel(**inputs) takes the complete unsharded inputs
(x: (8, 512, 32, 32), gamma: (1, 512, 1, 1), w_qkv: (1536, 512),
w_out: (512, 512)) and returns the full (8, 512, 32, 32) output.

Sharding: data-parallel over batch — core b computes batch element b
entirely on-chip. No collectives.

Per-core math (b fixed), [channel(part), spatial(free)] layout:
  xn = LayerNorm_c(x) * gamma          (stats via ones-matmul rows on PE)
  q,k = W_qk @ xn ; vT = xn^T W_v^T    (vT quantized to fp8e4, DoubleRow
                                        pair layout [j,pair,h,sub,80])
  per head: simT = k^T q (bf16); expT = fp8e4(exp(simT*s - 4ln2))
            (bias cancels in normalization; max exp ~25 << 240)
  out_aug = DoubleRow fp8 matmul [v;1] @ exp -> numerator + denominator
  att = numerator * broadcast(1/denominator)
  y = w_out @ att + x

Structure tuned against the HAM clock gate: exp on ACT (1.2 GHz fixed) is
the pacing engine, so ACT runs exp ONLY (evictions on DVE), the qk
projection is folded into the per-head pipeline to keep PE dense, and the
attn@v matmul runs fp8 DoubleRow (half the PE stream cycles of bf16).
"""

import numpy as np

C = 512
N = 1024
O3 = 1536
H = 8
DH = 64
EPS = 1e-5
SCALE = 64.0 ** -0.5
EXP_BIAS = -2.7725887222397811  # -4*ln2, cancels in softmax normalization
NCORES = 8

_CACHE = {}


def _build():
    import concourse.bacc as bacc
    import concourse.tile as tile
    from concourse import mybir
    from concourse.masks import make_identity

    f32 = mybir.dt.float32
    f32r = mybir.dt.float32r
    bf16 = mybir.dt.bfloat16
    fp8 = mybir.dt.float8e4
    AF = mybir.ActivationFunctionType
    OP = mybir.AluOpType
    DR = mybir.MatmulPerfMode.DoubleRow

    nc = bacc.Bacc("TRN2", target_bir_lowering=False, debug=False, num_devices=1)
    x_ap = nc.dram_tensor("x", [C, N], f32, kind="ExternalInput").ap()
    g_ap = nc.dram_tensor("gamma", [C], f32, kind="ExternalInput").ap()
    wqkv_ap = nc.dram_tensor("w_qkv", [O3, C], f32, kind="ExternalInput").ap()
    wout_ap = nc.dram_tensor("w_out", [C, C], f32, kind="ExternalInput").ap()
    y_ap = nc.dram_tensor("y", [C, N], f32, kind="ExternalOutput").ap()

    mm = nc.tensor.matmul

    with tile.TileContext(nc) as tc:
        with (
            tc.tile_pool(name="const", bufs=1) as const,
            tc.tile_pool(name="xin", bufs=1) as xin,
            tc.tile_pool(name="acts", bufs=1) as acts,
            tc.tile_pool(name="wTp", bufs=1) as wTp,
            tc.tile_pool(name="rows", bufs=1) as rows,
        ):
            ident = const.tile([128, 128], f32)
            make_identity(nc, ident)
            ones_f = const.tile([128, 1], f32)
            nc.vector.memset(ones_f, 1.0)
            ones_col = const.tile([128, 1], f32r)
            nc.scalar.copy(out=ones_col, in_=ones_f)
            # row operand for K=1 broadcast matmuls (base partition 0 only)
            onesr_f = const.tile([1, DH], f32)
            nc.vector.memset(onesr_f, 1.0)
            ones_row = const.tile([1, DH], f32r)
            nc.scalar.copy(out=ones_row, in_=onesr_f)
            gamma_f = const.tile([1, C], f32)
            nc.sync.dma_start(out=gamma_f, in_=g_ap[None, :])
            gamma_row = const.tile([1, C], f32r)
            nc.scalar.copy(out=gamma_row, in_=gamma_f)
            eps_col = const.tile([65, 1], f32)
            nc.vector.memset(eps_col, EPS)
            bias_col = const.tile([128, 1], f32)
            nc.vector.memset(bias_col, EXP_BIAS)

            # ---- load x ----
            x_sb = xin.tile([128, 4, N], f32)
            for t in range(4):
                nc.sync.dma_start(
                    out=x_sb[:, t, :], in_=x_ap[t * 128 : (t + 1) * 128, :]
                )

            # ---- persistent activation tiles ----
            qk_sb = acts.tile([128, 8, N], bf16)  # q: 0..3, k: 4..7
            # v^T in DoubleRow pair layout: [j, jt-pair, h, sub, c-aug(80)]
            # cols 0:64 = v, col 64 = 1 (denominator), 65:80 = 0 (pad)
            vT2 = acts.tile([128, 4, H, 2, 80], fp8)
            att_sb = acts.tile([128, 4, N], bf16)
            nc.gpsimd.memset(vT2, 0.0)
            nc.gpsimd.memset(vT2[:, :, :, :, DH : DH + 1], 1.0)

            # weights, transposed ([contraction-part, out-free])
            wqkT = wTp.tile([128, 4, 1024], bf16)
            wvT = wTp.tile([128, 4, C], bf16)
            woT = wTp.tile([128, 4, C], bf16)

            # stat rows, packed at 32-aligned partitions of shared tiles
            stA = rows.tile([97, N], f32)  # mean@0, msq@32, var@64, sd@96
            stB = rows.tile([97, N], f32)  # a@0, b@32
            st_ra = rows.tile([1, N], f32r)  # a (rounded), base 0
            st_rb = rows.tile([1, N], f32r)  # b (rounded), base 0
            mean_r, msq_r, var_r, sd_r = (
                stA[0:1, :],
                stA[32:33, :],
                stA[64:65, :],
                stA[96:97, :],
            )
            a_r, b_r = stB[0:1, :], stB[32:33, :]

            with tc.tile_pool(name="xnp", bufs=1) as xnp:
                xn_sb = xnp.tile([128, 4, N], bf16)

                # ============ phase A: W transpose + stats + xn ============
                with (
                    tc.tile_pool(name="wnat", bufs=3) as wnat,
                    tc.tile_pool(name="tmp", bufs=2) as tmp,
                ):
                    tp_stack = tc.tile_pool(name="tp_ps", bufs=2, space="PSUM")
                    tp_ps = tp_stack.__enter__()
                    st_stack = tc.tile_pool(name="st_ps", bufs=1, space="PSUM")
                    st_ps = st_stack.__enter__()
                    st_px = st_ps.tile([1, N], f32, tag="sx")
                    st_pq = st_ps.tile([1, N], f32, tag="sq")

                    # ---- stats rows: sum(x) fp32 matmul, sum(x^2) f32r ----
                    for t in range(4):
                        xsq = tmp.tile([128, N], f32r, tag="xsq")
                        nc.vector.tensor_mul(xsq, x_sb[:, t, :], x_sb[:, t, :])
                        for ch in range(2):
                            sl = slice(ch * 512, (ch + 1) * 512)
                            mm(
                                st_px[:, sl],
                                ones_f,
                                x_sb[:, t, sl],
                                start=(t == 0),
                                stop=(t == 3),
                            )
                            mm(
                                st_pq[:, sl],
                                ones_col,
                                xsq[:, sl],
                                start=(t == 0),
                                stop=(t == 3),
                            )

                    # evictions split DVE/ACT to balance phase-A load
                    tcount = [0]

                    def transp(dst, src):
                        ps = tp_ps.tile([128, 128], f32)
                        nc.tensor.transpose(ps, src, ident)
                        if tcount[0] % 2 == 0:
                            nc.vector.tensor_copy(dst, ps)
                        else:
                            nc.scalar.copy(out=dst, in_=ps)
                        tcount[0] += 1

                    # q,k weight tiles first: transposes interleave with the
                    # stats stream; wv/wout transposes are deferred into the
                    # xn window below to keep PE fed there.
                    wn_store = {}
                    for ot in range(12):
                        if ot < 8:
                            wn = wnat.tile([128, C], f32, tag="wn-%d" % (ot % 3))
                        else:
                            wn = wnat.tile([128, C], f32, tag="wnv-%d" % ot)
                            wn_store[ot] = wn
                        nc.sync.dma_start(
                            out=wn, in_=wqkv_ap[ot * 128 : (ot + 1) * 128, :]
                        )
                        if ot < 8:
                            for kt in range(4):
                                transp(
                                    wqkT[:, kt, ot * 128 : (ot + 1) * 128],
                                    wn[:, kt * 128 : (kt + 1) * 128],
                                )
                    for ot in range(4):
                        wn = wnat.tile([128, C], f32, tag="wno-%d" % ot)
                        wn_store[12 + ot] = wn
                        nc.sync.dma_start(
                            out=wn, in_=wout_ap[ot * 128 : (ot + 1) * 128, :]
                        )

                    nc.scalar.activation(mean_r, st_px, AF.Copy, scale=1.0 / C)
                    nc.vector.tensor_mul(msq_r, mean_r, mean_r)
                    nc.vector.scalar_tensor_tensor(
                        out=var_r,
                        in0=st_pq,
                        scalar=1.0 / C,
                        in1=msq_r,
                        op0=OP.mult,
                        op1=OP.subtract,
                    )
                    nc.scalar.activation(
                        sd_r, var_r, AF.Sqrt, bias=eps_col[64:65, :]
                    )
                    nc.vector.reciprocal(a_r, sd_r)
                    nc.vector.scalar_tensor_tensor(
                        out=b_r,
                        in0=mean_r,
                        scalar=-1.0,
                        in1=a_r,
                        op0=OP.mult,
                        op1=OP.mult,
                    )
                    nc.scalar.copy(out=st_ra, in_=a_r)
                    nc.scalar.copy(out=st_rb, in_=b_r)
                    st_stack.__exit__(None, None, None)

                    # ---- xn = x * (gamma (x) a) + (gamma (x) b) ----
                    # wv/wout transposes are interleaved here: they keep PE
                    # busy while the xn chain runs on DVE.
                    def transp_deferred(ot):
                        wn = wn_store.pop(ot)
                        for kt in range(4):
                            src = wn[:, kt * 128 : (kt + 1) * 128]
                            if ot < 12:
                                transp(
                                    wvT[:, kt, (ot - 8) * 128 : (ot - 7) * 128],
                                    src,
                                )
                            else:
                                transp(
                                    woT[:, kt, (ot - 12) * 128 : (ot - 11) * 128],
                                    src,
                                )

                    with tc.tile_pool(name="ab_ps", bufs=1, space="PSUM") as ab_ps:
                        for t in range(4):
                            ab = ab_ps.tile([128, 2, N], f32)
                            tsl = slice(t * 128, (t + 1) * 128)
                            for ch in range(2):
                                sl = slice(ch * 512, (ch + 1) * 512)
                                mm(ab[:, 0, sl], gamma_row[0:1, tsl], st_ra[:, sl])
                                mm(ab[:, 1, sl], gamma_row[0:1, tsl], st_rb[:, sl])
                            transp_deferred(8 + t)
                            for ch in range(2):
                                sl = slice(ch * 512, (ch + 1) * 512)
                                t1 = tmp.tile([128, 512], f32, tag="xnt")
                                nc.vector.tensor_mul(
                                    t1, x_sb[:, t, sl], ab[:, 0, sl]
                                )
                                nc.vector.tensor_add(
                                    xn_sb[:, t, sl], t1, ab[:, 1, sl]
                                )
                            transp_deferred(12 + t)
                    tp_stack.__exit__(None, None, None)

                # ======= phase BC: fused qkv projection + attention =======
                # Per-head pipeline: sim/exp(h) | attnv+normalize(h-1), with
                # qk projections and the v projection interleaved to keep
                # the PE stream dense (HAM stays un-throttled). ACT runs exp
                # only; DVE takes every PSUM eviction.
                with (
                    tc.tile_pool(name="expT", bufs=2) as expp,
                    tc.tile_pool(name="rrp", bufs=2) as rrp,
                    tc.tile_pool(name="oanp", bufs=2) as oanp,
                    tc.tile_pool(name="sim_ps", bufs=2, space="PSUM") as sim_ps,
                    tc.tile_pool(name="qk_ps", bufs=1, space="PSUM") as qk_psp,
                    tc.tile_pool(name="oa_ps", bufs=1, space="PSUM") as oa_psp,
                    tc.tile_pool(name="rb_ps", bufs=1, space="PSUM") as rb_psp,
                ):

                    def qkproj_group(ot, ch):
                        sl = slice(ch * 512, (ch + 1) * 512)
                        ps = qk_psp.tile([128, 512], f32, tag="qk")
                        for kt in range(4):
                            mm(
                                ps,
                                wqkT[:, kt, ot * 128 : (ot + 1) * 128],
                                xn_sb[:, kt, sl],
                                start=(kt == 0),
                                stop=(kt == 3),
                            )
                        nc.vector.tensor_copy(qk_sb[:, ot, sl], ps)

                    def qkproj(pair):  # q,k o-tiles for heads 2p, 2p+1
                        for ot in (pair, 4 + pair):
                            for ch in range(2):
                                qkproj_group(ot, ch)

                    def vproj(nt):
                        ps = qk_psp.tile([128, 512], f32, tag="qk")
                        for kt in range(4):
                            mm(
                                ps,
                                xn_sb[:, kt, nt * 128 : (nt + 1) * 128],
                                wvT[:, kt, :],
                                start=(kt == 0),
                                stop=(kt == 3),
                            )
                        nc.vector.tensor_copy(
                            vT2[:, nt // 2, :, nt % 2, 0:DH],
                            ps.rearrange("p (h d) -> p h d", h=H),
                        )

                    def sim_exp(h, e_t, jts):
                        hp = (h % 2) * DH
                        q_h = qk_sb[hp : hp + DH, h // 2, :]
                        k_h = qk_sb[hp : hp + DH, 4 + h // 2, :]
                        for jt in jts:
                            sim_t = sim_ps.tile([128, N], f32, tag="sim")
                            for ch in range(2):
                                sl = slice(ch * 512, (ch + 1) * 512)
                                mm(
                                    sim_t[:, sl],
                                    k_h[:, jt * 128 : (jt + 1) * 128],
                                    q_h[:, sl],
                                )
                            nc.scalar.activation(
                                e_t[:, jt // 2, jt % 2, :],
                                sim_t,
                                AF.Exp,
                                scale=SCALE,
                                bias=bias_col,
                            )

                    def attnv(hm, e_t):
                        oa = oa_psp.tile([128, N], f32, tag="oa")
                        for p in range(4):
                            for ch in range(2):
                                sl = slice(ch * 512, (ch + 1) * 512)
                                mm(
                                    oa[0:80, sl],
                                    vT2[:, p, hm, :, :],
                                    e_t[:, p, :, sl],
                                    start=(p == 0),
                                    stop=(p == 3),
                                    perf_mode=DR,
                                )
                        # reciprocal_approx_fast misreads PSUM operands:
                        # stage the denominator row through SBUF first.
                        rden_t = rrp.tile([1, N], f32, tag="rden")
                        nc.vector.tensor_copy(rden_t, oa[DH : DH + 1, :])
                        rrec_t = rrp.tile([1, N], f32, tag="rr")
                        nc.vector.reciprocal_approx_fast(out=rrec_t, in_=rden_t)
                        rd = rrp.tile([1, N], f32r, tag="rd")
                        nc.vector.tensor_copy(rd, rrec_t)
                        return oa, rd

                    def normalize(h2, oa, rd):
                        # DVE has a single PSUM read port: numerator must be
                        # in SBUF before the mult against the PSUM rb rows.
                        hp2 = (h2 % 2) * DH
                        oan = oanp.tile([DH, N], f32, tag="oan")
                        nc.vector.tensor_copy(oan, oa[0:DH, :])
                        for ch in range(2):
                            sl = slice(ch * 512, (ch + 1) * 512)
                            rb = rb_psp.tile([DH, 512], f32, tag="rb")
                            mm(rb, ones_row, rd[:, sl])
                            nc.vector.tensor_mul(
                                att_sb[hp2 : hp2 + DH, h2 // 2, sl],
                                oan[:, sl],
                                rb,
                            )

                    # ---- fill: head 0 sims interleaved with v projection ----
                    qkproj(0)
                    e_prev = expp.tile([128, 4, 2, N], fp8, tag="exp")
                    for jt in range(8):
                        sim_exp(0, e_prev, [jt])
                        vproj(jt)
                    qkproj(1)

                    # late qk projection groups, two per iteration, emitted
                    # between sim pairs so the in-order PE queue never stalls
                    # at the sim double-buffer boundary.
                    qk_sched = {
                        2: [(2, 0), (6, 0)],
                        3: [(2, 1), (6, 1)],
                        4: [(3, 0), (7, 0)],
                        5: [(3, 1), (7, 1)],
                    }

                    # ---- steady state ----
                    for it in range(1, H + 1):
                        h, hm = it, it - 1
                        groups = qk_sched.get(it, [])
                        e_t = None
                        if h < H:
                            e_t = expp.tile([128, 4, 2, N], fp8, tag="exp")
                            sim_exp(h, e_t, [0, 1])
                        oa, rd = attnv(hm, e_prev)
                        if h < H:
                            sim_exp(h, e_t, [2, 3])
                            if groups:
                                qkproj_group(*groups[0])
                            sim_exp(h, e_t, [4, 5])
                            if groups:
                                qkproj_group(*groups[1])
                            sim_exp(h, e_t, [6, 7])
                        normalize(hm, oa, rd)
                        e_prev = e_t

            # ============ phase D: out projection + residual ============
            with (
                tc.tile_pool(name="evict", bufs=3) as evict,
                tc.tile_pool(name="z_ps", bufs=3, space="PSUM") as z_ps,
            ):
                for ot in range(4):
                    for ch in range(2):
                        sl = slice(ch * 512, (ch + 1) * 512)
                        ps = z_ps.tile([128, 512], f32)
                        for kt in range(4):
                            mm(
                                ps,
                                woT[:, kt, ot * 128 : (ot + 1) * 128],
                                att_sb[:, kt, sl],
                                start=(kt == 0),
                                stop=(kt == 3),
                            )
                        yt = evict.tile([128, 512], f32)
                        nc.vector.tensor_add(yt, ps, x_sb[:, ot, sl])
                        nc.sync.dma_start(
                            out=y_ap[ot * 128 : (ot + 1) * 128, sl], in_=yt
                        )

    nc.compile()
    return nc


def _get_nc():
    if "nc" not in _CACHE:
        _CACHE["nc"] = _build()
    return _CACHE["nc"]


def kernel(x, gamma, w_qkv, w_out):
    from concourse.bass_utils import run_bass_kernel_spmd

    x = np.ascontiguousarray(x, dtype=np.float32)
    gamma_f = np.ascontiguousarray(gamma, dtype=np.float32).reshape(C)
    wq = np.ascontiguousarray(w_qkv, dtype=np.float32)
    wo = np.ascontiguousarray(w_out, dtype=np.float32)

    nc = _get_nc()
    in_maps = [
        {
            "x": x[b].reshape(C, N).copy(),
            "gamma": gamma_f,
            "w_qkv": wq,
            "w_out": wo,
        }
        for b in range(NCORES)
    ]
    res = run_bass_kernel_spmd(nc, in_maps, core_ids=list(range(NCORES)))
    out = np.stack(
        [res.results[b]["y"].reshape(C, 32, 32) for b in range(NCORES)], axis=0
    )
    return out.astype(np.float32)


# revision 14
# speedup vs baseline: 1.0621x; 1.0306x over previous
"""ConvAttention Trainium2 kernel.

Full-input contract: kernel(**inputs) takes the complete unsharded inputs
(x: (8, 512, 32, 32), gamma: (1, 512, 1, 1), w_qkv: (1536, 512),
w_out: (512, 512)) and returns the full (8, 512, 32, 32) output.

Sharding: data-parallel over batch — core b computes batch element b
entirely on-chip. No collectives.

Per-core math (b fixed), [channel(part), spatial(free)] layout:
  xn = LayerNorm_c(x) * gamma          (stats via ones-matmul rows on PE)
  q,k = W_qk @ xn ; vT = xn^T W_v^T    (vT quantized to fp8e4, DoubleRow
                                        pair layout [j,pair,h,sub,80])
  per head: simT = k^T q (bf16); expT = fp8e4(exp(simT*s - 4ln2))
            (bias cancels in normalization; max exp ~25 << 240)
  out_aug = DoubleRow fp8 matmul [v;1] @ exp -> numerator + denominator
  att = numerator * broadcast(1/denominator)
  y = w_out @ att + x

Structure tuned against the HAM clock gate: exp on ACT (1.2 GHz fixed) is
the pacing engine, so ACT runs exp ONLY (evictions on DVE), the qk
projection is folded into the per-head pipeline to keep PE dense, and the
attn@v matmul runs fp8 DoubleRow (half the PE stream cycles of bf16).
"""

import numpy as np

C = 512
N = 1024
O3 = 1536
H = 8
DH = 64
EPS = 1e-5
SCALE = 64.0 ** -0.5
EXP_BIAS = -2.7725887222397811  # -4*ln2, cancels in softmax normalization
NCORES = 8

_CACHE = {}


def _build():
    import concourse.bacc as bacc
    import concourse.tile as tile
    from concourse import mybir
    from concourse.masks import make_identity

    f32 = mybir.dt.float32
    f32r = mybir.dt.float32r
    bf16 = mybir.dt.bfloat16
    fp8 = mybir.dt.float8e4
    AF = mybir.ActivationFunctionType
    OP = mybir.AluOpType
    DR = mybir.MatmulPerfMode.DoubleRow

    nc = bacc.Bacc("TRN2", target_bir_lowering=False, debug=False, num_devices=1)
    x_ap = nc.dram_tensor("x", [C, N], f32, kind="ExternalInput").ap()
    g_ap = nc.dram_tensor("gamma", [C], f32, kind="ExternalInput").ap()
    wqkv_ap = nc.dram_tensor("w_qkv", [O3, C], f32, kind="ExternalInput").ap()
    wout_ap = nc.dram_tensor("w_out", [C, C], f32, kind="ExternalInput").ap()
    y_ap = nc.dram_tensor("y", [C, N], f32, kind="ExternalOutput").ap()

    mm = nc.tensor.matmul

    with tile.TileContext(nc) as tc:
        with (
            tc.tile_pool(name="const", bufs=1) as const,
            tc.tile_pool(name="xin", bufs=1) as xin,
            tc.tile_pool(name="acts", bufs=1) as acts,
            tc.tile_pool(name="wTp", bufs=1) as wTp,
            tc.tile_pool(name="rows", bufs=1) as rows,
        ):
            ident = const.tile([128, 128], f32)
            make_identity(nc, ident)
            ones_f = const.tile([128, 1], f32)
            nc.vector.memset(ones_f, 1.0)
            ones_col = const.tile([128, 1], f32r)
            nc.scalar.copy(out=ones_col, in_=ones_f)
            # row operand for K=1 broadcast matmuls (base partition 0 only)
            onesr_f = const.tile([1, DH], f32)
            nc.vector.memset(onesr_f, 1.0)
            ones_row = const.tile([1, DH], f32r)
            nc.scalar.copy(out=ones_row, in_=onesr_f)
            gamma_f = const.tile([1, C], f32)
            nc.sync.dma_start(out=gamma_f, in_=g_ap[None, :])
            gamma_row = const.tile([1, C], f32r)
            nc.scalar.copy(out=gamma_row, in_=gamma_f)
            eps_col = const.tile([65, 1], f32)
            nc.vector.memset(eps_col, EPS)
            bias_col = const.tile([128, 1], f32)
            nc.vector.memset(bias_col, EXP_BIAS)

            # ---- load x ----
            x_sb = xin.tile([128, 4, N], f32)
            for t in range(4):
                nc.sync.dma_start(
                    out=x_sb[:, t, :], in_=x_ap[t * 128 : (t + 1) * 128, :]
                )

            # ---- persistent activation tiles ----
            qk_sb = acts.tile([128, 8, N], bf16)  # q: 0..3, k: 4..7
            # v^T in DoubleRow pair layout: [j, jt-pair, h, sub, c-aug(80)]
            # cols 0:64 = v, col 64 = 1 (denominator), 65:80 = 0 (pad)
            vT2 = acts.tile([128, 4, H, 2, 80], fp8)
            att_sb = acts.tile([128, 4, N], bf16)
            nc.gpsimd.memset(vT2, 0.0)
            nc.gpsimd.memset(vT2[:, :, :, :, DH : DH + 1], 1.0)

            # weights, transposed ([contraction-part, out-free])
            wqkT = wTp.tile([128, 4, 1024], bf16)
            wvT = wTp.tile([128, 4, C], bf16)
            woT = wTp.tile([128, 4, C], bf16)

            # stat rows, packed at 32-aligned partitions of shared tiles
            stA = rows.tile([97, N], f32)  # mean@0, msq@32, var@64, sd@96
            stB = rows.tile([97, N], f32)  # a@0, b@32
            st_ra = rows.tile([1, N], f32r)  # a (rounded), base 0
            st_rb = rows.tile([1, N], f32r)  # b (rounded), base 0
            mean_r, msq_r, var_r, sd_r = (
                stA[0:1, :],
                stA[32:33, :],
                stA[64:65, :],
                stA[96:97, :],
            )
            a_r, b_r = stB[0:1, :], stB[32:33, :]

            with tc.tile_pool(name="xnp", bufs=1) as xnp:
                xn_sb = xnp.tile([128, 4, N], bf16)

                # ============ phase A: W transpose + stats + xn ============
                with (
                    tc.tile_pool(name="wnat", bufs=3) as wnat,
                    tc.tile_pool(name="tmp", bufs=2) as tmp,
                ):
                    tp_stack = tc.tile_pool(name="tp_ps", bufs=4, space="PSUM")
                    tp_ps = tp_stack.__enter__()
                    st_stack = tc.tile_pool(name="st_ps", bufs=1, space="PSUM")
                    st_ps = st_stack.__enter__()
                    st_px = st_ps.tile([1, N], f32, tag="sx")
                    st_pq = st_ps.tile([1, N], f32, tag="sq")

                    # ---- stats rows: sum(x) fp32 matmul, sum(x^2) f32r ----
                    for t in range(4):
                        xsq = tmp.tile([128, N], f32r, tag="xsq")
                        nc.vector.tensor_mul(xsq, x_sb[:, t, :], x_sb[:, t, :])
                        for ch in range(2):
                            sl = slice(ch * 512, (ch + 1) * 512)
                            mm(
                                st_px[:, sl],
                                ones_f,
                                x_sb[:, t, sl],
                                start=(t == 0),
                                stop=(t == 3),
                            )
                            mm(
                                st_pq[:, sl],
                                ones_col,
                                xsq[:, sl],
                                start=(t == 0),
                                stop=(t == 3),
                            )

                    # evictions split DVE/ACT to balance phase-A load
                    tcount = [0]

                    def transp(dst, src):
                        ps = tp_ps.tile([128, 128], f32)
                        nc.tensor.transpose(ps, src, ident)
                        if tcount[0] % 2 == 0:
                            nc.vector.tensor_copy(dst, ps)
                        else:
                            nc.scalar.copy(out=dst, in_=ps)
                        tcount[0] += 1

                    # q,k weight tiles first: transposes interleave with the
                    # stats stream; wv/wout transposes are deferred into the
                    # xn window below to keep PE fed there.
                    wn_store = {}
                    for ot in range(12):
                        if ot < 8:
                            wn = wnat.tile([128, C], f32, tag="wn-%d" % (ot % 3))
                        else:
                            wn = wnat.tile([128, C], f32, tag="wnv-%d" % ot)
                            wn_store[ot] = wn
                        nc.sync.dma_start(
                            out=wn, in_=wqkv_ap[ot * 128 : (ot + 1) * 128, :]
                        )
                        if ot < 8:
                            for kt in range(4):
                                transp(
                                    wqkT[:, kt, ot * 128 : (ot + 1) * 128],
                                    wn[:, kt * 128 : (kt + 1) * 128],
                                )
                    for ot in range(4):
                        wn = wnat.tile([128, C], f32, tag="wno-%d" % ot)
                        wn_store[12 + ot] = wn
                        nc.sync.dma_start(
                            out=wn, in_=wout_ap[ot * 128 : (ot + 1) * 128, :]
                        )

                    nc.scalar.activation(mean_r, st_px, AF.Copy, scale=1.0 / C)
                    nc.vector.tensor_mul(msq_r, mean_r, mean_r)
                    nc.vector.scalar_tensor_tensor(
                        out=var_r,
                        in0=st_pq,
                        scalar=1.0 / C,
                        in1=msq_r,
                        op0=OP.mult,
                        op1=OP.subtract,
                    )
                    nc.scalar.activation(
                        sd_r, var_r, AF.Sqrt, bias=eps_col[64:65, :]
                    )
                    nc.vector.reciprocal(a_r, sd_r)
                    nc.vector.scalar_tensor_tensor(
                        out=b_r,
                        in0=mean_r,
                        scalar=-1.0,
                        in1=a_r,
                        op0=OP.mult,
                        op1=OP.mult,
                    )
                    nc.scalar.copy(out=st_ra, in_=a_r)
                    nc.scalar.copy(out=st_rb, in_=b_r)
                    st_stack.__exit__(None, None, None)

                    # ---- xn = x * (gamma (x) a) + (gamma (x) b) ----
                    # wv/wout transposes are interleaved here: they keep PE
                    # busy while the xn chain runs on DVE.
                    def transp_deferred(ot):
                        wn = wn_store.pop(ot)
                        for kt in range(4):
                            src = wn[:, kt * 128 : (kt + 1) * 128]
                            if ot < 12:
                                transp(
                                    wvT[:, kt, (ot - 8) * 128 : (ot - 7) * 128],
                                    src,
                                )
                            else:
                                transp(
                                    woT[:, kt, (ot - 12) * 128 : (ot - 11) * 128],
                                    src,
                                )

                    with tc.tile_pool(name="ab_ps", bufs=1, space="PSUM") as ab_ps:
                        for t in range(4):
                            ab = ab_ps.tile([128, 2, N], f32)
                            tsl = slice(t * 128, (t + 1) * 128)
                            for ch in range(2):
                                sl = slice(ch * 512, (ch + 1) * 512)
                                mm(ab[:, 0, sl], gamma_row[0:1, tsl], st_ra[:, sl])
                                mm(ab[:, 1, sl], gamma_row[0:1, tsl], st_rb[:, sl])
                            transp_deferred(8 + t)
                            for ch in range(2):
                                sl = slice(ch * 512, (ch + 1) * 512)
                                t1 = tmp.tile([128, 512], f32, tag="xnt")
                                nc.vector.tensor_mul(
                                    t1, x_sb[:, t, sl], ab[:, 0, sl]
                                )
                                nc.vector.tensor_add(
                                    xn_sb[:, t, sl], t1, ab[:, 1, sl]
                                )
                            transp_deferred(12 + t)
                    tp_stack.__exit__(None, None, None)

                # ======= phase BC: fused qkv projection + attention =======
                # Per-head pipeline: sim/exp(h) | attnv+normalize(h-1), with
                # qk projections and the v projection interleaved to keep
                # the PE stream dense (HAM stays un-throttled). ACT runs exp
                # only; DVE takes every PSUM eviction.
                with (
                    tc.tile_pool(name="expT", bufs=2) as expp,
                    tc.tile_pool(name="rrp", bufs=2) as rrp,
                    tc.tile_pool(name="oanp", bufs=2) as oanp,
                    tc.tile_pool(name="sim_ps", bufs=2, space="PSUM") as sim_ps,
                    tc.tile_pool(name="qk_ps", bufs=1, space="PSUM") as qk_psp,
                    tc.tile_pool(name="oa_ps", bufs=1, space="PSUM") as oa_psp,
                    tc.tile_pool(name="rb_ps", bufs=1, space="PSUM") as rb_psp,
                ):

                    def qkproj_group(ot, ch):
                        sl = slice(ch * 512, (ch + 1) * 512)
                        ps = qk_psp.tile([128, 512], f32, tag="qk")
                        for kt in range(4):
                            mm(
                                ps,
                                wqkT[:, kt, ot * 128 : (ot + 1) * 128],
                                xn_sb[:, kt, sl],
                                start=(kt == 0),
                                stop=(kt == 3),
                            )
                        nc.vector.tensor_copy(qk_sb[:, ot, sl], ps)

                    def qkproj(pair):  # q,k o-tiles for heads 2p, 2p+1
                        for ot in (pair, 4 + pair):
                            for ch in range(2):
                                qkproj_group(ot, ch)

                    def vproj(nt):
                        ps = qk_psp.tile([128, 512], f32, tag="qk")
                        for kt in range(4):
                            mm(
                                ps,
                                xn_sb[:, kt, nt * 128 : (nt + 1) * 128],
                                wvT[:, kt, :],
                                start=(kt == 0),
                                stop=(kt == 3),
                            )
                        nc.vector.tensor_copy(
                            vT2[:, nt // 2, :, nt % 2, 0:DH],
                            ps.rearrange("p (h d) -> p h d", h=H),
                        )

                    def sim_exp(h, e_t, jts):
                        hp = (h % 2) * DH
                        q_h = qk_sb[hp : hp + DH, h // 2, :]
                        k_h = qk_sb[hp : hp + DH, 4 + h // 2, :]
                        for jt in jts:
                            sim_t = sim_ps.tile([128, N], f32, tag="sim")
                            for ch in range(2):
                                sl = slice(ch * 512, (ch + 1) * 512)
                                mm(
                                    sim_t[:, sl],
                                    k_h[:, jt * 128 : (jt + 1) * 128],
                                    q_h[:, sl],
                                )
                            nc.scalar.activation(
                                e_t[:, jt // 2, jt % 2, :],
                                sim_t,
                                AF.Exp,
                                scale=SCALE,
                                bias=bias_col,
                            )

                    def attnv(hm, e_t):
                        oa = oa_psp.tile([128, N], f32, tag="oa")
                        for p in range(4):
                            for ch in range(2):
                                sl = slice(ch * 512, (ch + 1) * 512)
                                mm(
                                    oa[0:80, sl],
                                    vT2[:, p, hm, :, :],
                                    e_t[:, p, :, sl],
                                    start=(p == 0),
                                    stop=(p == 3),
                                    perf_mode=DR,
                                )
                        # reciprocal_approx_fast misreads PSUM operands:
                        # stage the denominator row through SBUF first.
                        rden_t = rrp.tile([1, N], f32, tag="rden")
                        nc.vector.tensor_copy(rden_t, oa[DH : DH + 1, :])
                        rrec_t = rrp.tile([1, N], f32, tag="rr")
                        nc.vector.reciprocal_approx_fast(out=rrec_t, in_=rden_t)
                        rd = rrp.tile([1, N], f32r, tag="rd")
                        nc.vector.tensor_copy(rd, rrec_t)
                        return oa, rd

                    def normalize(h2, oa, rd):
                        # DVE has a single PSUM read port: numerator must be
                        # in SBUF before the mult against the PSUM rb rows.
                        hp2 = (h2 % 2) * DH
                        oan = oanp.tile([DH, N], f32, tag="oan")
                        nc.vector.tensor_copy(oan, oa[0:DH, :])
                        for ch in range(2):
                            sl = slice(ch * 512, (ch + 1) * 512)
                            rb = rb_psp.tile([DH, 512], f32, tag="rb")
                            mm(rb, ones_row, rd[:, sl])
                            nc.vector.tensor_mul(
                                att_sb[hp2 : hp2 + DH, h2 // 2, sl],
                                oan[:, sl],
                                rb,
                            )

                    # ---- fill: head 0 sims interleaved with v projection ----
                    qkproj(0)
                    e_prev = expp.tile([128, 4, 2, N], fp8, tag="exp")
                    for jt in range(8):
                        sim_exp(0, e_prev, [jt])
                        vproj(jt)
                    qkproj(1)

                    # late qk projection groups, two per iteration, emitted
                    # between sim pairs so the in-order PE queue never stalls
                    # at the sim double-buffer boundary.
                    qk_sched = {
                        2: [(2, 0), (6, 0)],
                        3: [(2, 1), (6, 1)],
                        4: [(3, 0), (7, 0)],
                        5: [(3, 1), (7, 1)],
                    }

                    # ---- steady state ----
                    for it in range(1, H + 1):
                        h, hm = it, it - 1
                        groups = qk_sched.get(it, [])
                        e_t = None
                        if h < H:
                            e_t = expp.tile([128, 4, 2, N], fp8, tag="exp")
                            sim_exp(h, e_t, [0, 1])
                        oa, rd = attnv(hm, e_prev)
                        if h < H:
                            sim_exp(h, e_t, [2, 3])
                            if groups:
                                qkproj_group(*groups[0])
                            sim_exp(h, e_t, [4, 5])
                            if groups:
                                qkproj_group(*groups[1])
                            sim_exp(h, e_t, [6, 7])
                        normalize(hm, oa, rd)
                        e_prev = e_t

            # ============ phase D: out projection + residual ============
            with (
                tc.tile_pool(name="evict", bufs=3) as evict,
                tc.tile_pool(name="z_ps", bufs=3, space="PSUM") as z_ps,
            ):
                for ot in range(4):
                    for ch in range(2):
                        sl = slice(ch * 512, (ch + 1) * 512)
                        ps = z_ps.tile([128, 512], f32)
                        for kt in range(4):
                            mm(
                                ps,
                                woT[:, kt, ot * 128 : (ot + 1) * 128],
                                att_sb[:, kt, sl],
                                start=(kt == 0),
                                stop=(kt == 3),
                            )
                        yt = evict.tile([128, 512], f32)
                        nc.vector.tensor_add(yt, ps, x_sb[:, ot, sl])
                        nc.sync.dma_start(
                            out=y_ap[ot * 128 : (ot + 1) * 128, sl], in_=yt
                        )

    nc.compile()
    return nc


def _get_nc():
    if "nc" not in _CACHE:
        _CACHE["nc"] = _build()
    return _CACHE["nc"]


def kernel(x, gamma, w_qkv, w_out):
    from concourse.bass_utils import run_bass_kernel_spmd

    x = np.ascontiguousarray(x, dtype=np.float32)
    gamma_f = np.ascontiguousarray(gamma, dtype=np.float32).reshape(C)
    wq = np.ascontiguousarray(w_qkv, dtype=np.float32)
    wo = np.ascontiguousarray(w_out, dtype=np.float32)

    nc = _get_nc()
    in_maps = [
        {
            "x": x[b].reshape(C, N).copy(),
            "gamma": gamma_f,
            "w_qkv": wq,
            "w_out": wo,
        }
        for b in range(NCORES)
    ]
    res = run_bass_kernel_spmd(nc, in_maps, core_ids=list(range(NCORES)))
    out = np.stack(
        [res.results[b]["y"].reshape(C, 32, 32) for b in range(NCORES)], axis=0
    )
    return out.astype(np.float32)
